# revision 17
# baseline (speedup 1.0000x reference)
"""Trainium2 Bass kernel for per-position grouped-query attention.

Reference computation (B=4, S=4096, HID=2048, H=16, G=4, D=128, KV=512):
    q = x @ Wq + bq ; k = x @ Wk + bk ; v = x @ Wv + bv
    scores[t,h,g] = <q[t,h,:], k[t,g,:]> / sqrt(D)     (same-position only)
    probs = softmax_g(scores)
    o[t,h,:] = sum_g probs[t,h,g] * v[t,g,:]
    y = o @ Wo + bo

Strategy: data-parallel over the 16384 flattened tokens -> 2048 tokens/core
on 8 cores, all weights replicated, no collectives.  Per core the kernel is
PE-bound (~43 GFLOP bf16 -> 546us matmul floor @78.6TF/s).  The fast path:
  - x arrives host-pre-transposed per token tile (no PE transposes on the
    input side); QKV matmuls use x-tile blocks as the stationary operand.
  - attention middle: fused mult+accum score dots + weighted-sum chains on
    DVE, PSUM->SBUF casts + exp on ACT - all hidden under the PE stream.
  - O^T comes from the DMA xbar (dma_start_transpose), so the PE runs
    matmuls only; Wo matmuls consume each tile 3 pipeline steps later and
    y goes out via ACT copy + DMA.
  - weight DMAs are emitted in tile-0's consumption order (two halves of
    three interleaved psum group chains), so the DMA-bound startup overlaps
    the first two tiles' compute.
"""

import os
import sys

import numpy as np

sys.path.insert(0, "/opt/trn_rl_repo")

import ml_dtypes  # noqa: E402
from contextlib import ExitStack  # noqa: E402

import concourse.bass as bass  # noqa: E402
import concourse.bacc as bacc  # noqa: E402
import concourse.mybir as mybir  # noqa: E402
import concourse.tile as tile  # noqa: E402
from concourse.bass import ds  # noqa: E402
from concourse.bass_utils import run_bass_kernel_spmd  # noqa: E402
from concourse.masks import make_identity  # noqa: E402

B, S, HID = 4, 4096, 2048
H, G = 16, 4
D = HID // H          # 128
KV = HID * G // H     # 512
NCORES = 8
NTOK = B * S          # 16384
TPC = NTOK // NCORES  # 2048 tokens per core
P = 128
NTT = TPC // P        # 16 token tiles per core
NI = HID // P         # 16 input-feature blocks
SCALE = 1.0 / float(np.sqrt(D))

BF16 = mybir.dt.bfloat16
F32 = mybir.dt.float32

_cache = {}


def _build_fast() -> bass.Bass:
    """No-bias fast path."""
    nc = bacc.Bacc("TRN2")
    # xt: host-pretransposed per token tile: row (t*128+p), col (i*128+tok)
    # holds x[t*128+tok, i*128+p]  -> per tile a plain [128, 2048] slice whose
    # block i is the lhsT [feat-in-block, token] for the QKV matmuls.
    xt = nc.dram_tensor("xt", [TPC, HID], BF16, kind="ExternalInput")
    wq = nc.dram_tensor("wq", [HID, HID], BF16, kind="ExternalInput")
    wk = nc.dram_tensor("wk", [HID, KV], BF16, kind="ExternalInput")
    wv = nc.dram_tensor("wv", [HID, KV], BF16, kind="ExternalInput")
    wo = nc.dram_tensor("wo", [HID, HID], BF16, kind="ExternalInput")
    y = nc.dram_tensor("y", [TPC, HID], F32, kind="ExternalOutput")

    with tile.TileContext(nc) as tc, ExitStack() as ctx:
        w_pool = ctx.enter_context(tc.tile_pool(name="w", bufs=1))
        xt_pool = ctx.enter_context(tc.tile_pool(name="xt", bufs=2))
        ysb_pool = ctx.enter_context(tc.tile_pool(name="ysb", bufs=2))
        qkv_ps_pool = ctx.enter_context(
            tc.tile_pool(name="qkvps", bufs=3, space="PSUM"))
        y_ps_pool = ctx.enter_context(tc.tile_pool(name="yps", bufs=3, space="PSUM"))
        qsb_pool = ctx.enter_context(tc.tile_pool(name="qsb", bufs=2))
        sm_pool = ctx.enter_context(tc.tile_pool(name="sm", bufs=2))
        wt_pool = ctx.enter_context(tc.tile_pool(name="wt", bufs=2))
        obf_pool = ctx.enter_context(tc.tile_pool(name="obf", bufs=2))
        ot_pool = ctx.enter_context(tc.tile_pool(name="ot", bufs=2))

        xt_sb = [None] * NTT
        obf_sb = [None] * NTT
        ot_sb = [None] * NTT

        def load_x(t):
            xt_sb[t] = xt_pool.tile([P, HID], BF16, name="xt", tag="xt")
            nc.sync.dma_start(xt_sb[t][:], xt[t * P:(t + 1) * P, :])

        # DMA emission order is the serial-DMA schedule: x0, then the qkv
        # weights interleaved in tile-0 consumption order (two halves of
        # three 512-col groups each), x1, x2, then wo (needed 3 tiles in).
        load_x(0)
        wqa = w_pool.tile([P, NI * HID], BF16, tag="wqa", name="wqa")
        wka = w_pool.tile([P, NI * KV], BF16, tag="wka", name="wka")
        wva = w_pool.tile([P, NI * KV], BF16, tag="wva", name="wva")
        woa = w_pool.tile([P, NI * HID], BF16, tag="woa", name="woa")
        wq_sb = [wqa[:, i * HID:(i + 1) * HID] for i in range(NI)]
        wk_sb = [wka[:, i * KV:(i + 1) * KV] for i in range(NI)]
        wv_sb = [wva[:, i * KV:(i + 1) * KV] for i in range(NI)]
        wo_sb = [woa[:, i * HID:(i + 1) * HID] for i in range(NI)]
        CB = 4  # weight blocks per DMA chunk

        def wchunk(dst_all, src, j, c0, c1, w_):
            # blocks j*CB..(j+1)*CB-1, cols c0:c1, one DMA
            nc.sync.dma_start(
                dst_all[:, j * CB * w_:(j + 1) * CB * w_]
                .rearrange("p (i c) -> p i c", c=w_)[:, :, c0:c1],
                src[j * CB * P:(j + 1) * CB * P, c0:c1]
                .rearrange("(i p) c -> p i c", p=P),
            )

        for j in range(NI // CB):
            wchunk(wqa, wq, j, 0, 1024, HID)
            wchunk(wka, wk, j, 0, KV, KV)
        load_x(1)
        for j in range(NI // CB):
            wchunk(wqa, wq, j, 1024, 2048, HID)
            wchunk(wva, wv, j, 0, KV, KV)
        load_x(2)
        for j in range(NI // CB):
            wchunk(woa, wo, j, 0, HID, HID)

        for t in range(NTT + 3):
            if 1 <= t and t + 3 <= NTT:  # x3..x15 stream from the loop
                load_x(t + 2)

            if t < NTT:
                xts = xt_sb[t]
                qsb = qsb_pool.tile([P, HID], BF16, tag="q")
                ksb = qsb_pool.tile([P, KV], BF16, tag="k")
                vsb = qsb_pool.tile([P, KV], BF16, tag="v")

                def qkv_copy(s_, ps):
                    if s_ < 4:
                        nc.scalar.copy(qsb[:, s_ * 512:(s_ + 1) * 512], ps[:])
                    elif s_ == 4:
                        nc.scalar.copy(ksb[:], ps[:])
                    else:
                        nc.scalar.copy(vsb[:], ps[:])

                if t == 0:
                    # consume weights in DMA-arrival order: i-major, three
                    # simultaneous psum groups per half
                    for half in range(2):
                        svals = (0, 1, 4) if half == 0 else (2, 3, 5)
                        pss = []
                        for s_ in svals:
                            ps = qkv_ps_pool.tile([P, 512], F32, name="ps", tag="ps")
                            pss.append(ps)
                        for i in range(NI):
                            for ps, s_ in zip(pss, svals):
                                if s_ < 4:
                                    rhs = wq_sb[i][:, s_ * 512:(s_ + 1) * 512]
                                elif s_ == 4:
                                    rhs = wk_sb[i][:]
                                else:
                                    rhs = wv_sb[i][:]
                                nc.tensor.matmul(
                                    ps[:], xts[:, i * P:(i + 1) * P], rhs,
                                    start=(i == 0), stop=(i == NI - 1),
                                )
                        for ps, s_ in zip(pss, svals):
                            qkv_copy(s_, ps)
                else:
                    # steady state: 3 pair-interleaved group chains per tile;
                    # each xt block is stationary for 2 back-to-back matmuls
                    for sa, sb in ((0, 4), (1, 2), (3, 5)):
                        psa = qkv_ps_pool.tile([P, 512], F32, name="ps", tag="ps")
                        psb = qkv_ps_pool.tile([P, 512], F32, name="ps", tag="ps")
                        for i in range(NI):
                            for ps, s_ in ((psa, sa), (psb, sb)):
                                if s_ < 4:
                                    rhs = wq_sb[i][:, s_ * 512:(s_ + 1) * 512]
                                elif s_ == 4:
                                    rhs = wk_sb[i][:]
                                else:
                                    rhs = wv_sb[i][:]
                                nc.tensor.matmul(
                                    ps[:], xts[:, i * P:(i + 1) * P], rhs,
                                    start=(i == 0), stop=(i == NI - 1),
                                )
                        qkv_copy(sa, psa)
                        qkv_copy(sb, psb)

                # --- attention middle, token-major ---
                sc = sm_pool.tile([P, H * G], F32, tag="sc")
                ex = sm_pool.tile([P, H * G], F32, tag="ex")
                dn = sm_pool.tile([P, H], F32, tag="dn")
                rc = sm_pool.tile([P, H], F32, tag="rc")
                pf = sm_pool.tile([P, H * G], F32, tag="pf")
                junk = sm_pool.tile([P, D], BF16, tag="junk")

                # raw scores sc[t,(h,g)] = <q_h, k_g>  (fused mult+reduce, DVE)
                for h in range(H):
                    for g in range(G):
                        nc.vector.scalar_tensor_tensor(
                            junk[:],
                            qsb[:, h * D:(h + 1) * D],
                            1.0,
                            ksb[:, g * D:(g + 1) * D],
                            op0=mybir.AluOpType.mult,
                            op1=mybir.AluOpType.mult,
                            accum_out=sc[:, ds(h * G + g, 1)],
                        )

                # softmax over g (1/sqrt(D) folded into exp's scale)
                nc.scalar.activation(
                    ex[:], sc[:], mybir.ActivationFunctionType.Exp, scale=SCALE)
                nc.vector.reduce_sum(
                    dn[:], ex[:].rearrange("p (h g) -> p h g", g=G),
                    axis=mybir.AxisListType.X,
                )
                nc.vector.reciprocal(rc[:], dn[:])
                nc.vector.scalar_tensor_tensor(
                    pf[:].rearrange("p (h g) -> p h g", g=G),
                    ex[:].rearrange("p (h g) -> p h g", g=G),
                    1.0,
                    rc[:].unsqueeze(2).broadcast_to((P, H, G)),
                    op0=mybir.AluOpType.mult, op1=mybir.AluOpType.mult,
                )

                # o[t,(h,d)] = sum_g p[t,(h,g)] * v[t,(g,d)]  (DVE chain)
                obf = obf_pool.tile([P, HID], BF16, name="obf", tag="obf")
                obf_sb[t] = obf
                ta = wt_pool.tile([P, D], BF16, tag="ta")
                tb = wt_pool.tile([P, D], BF16, tag="tb")
                ab = [ta, tb]
                for h in range(H):
                    nc.vector.tensor_scalar_mul(
                        ab[0][:], vsb[:, 0:D], pf[:, ds(h * G, 1)])
                    for g in range(1, G):
                        dst = obf[:, h * D:(h + 1) * D] if g == G - 1 else ab[g % 2][:]
                        nc.vector.scalar_tensor_tensor(
                            dst,
                            vsb[:, g * D:(g + 1) * D],
                            pf[:, ds(h * G + g, 1)],
                            ab[(g - 1) % 2][:],
                            op0=mybir.AluOpType.mult,
                            op1=mybir.AluOpType.add,
                        )

            # Wo matmuls + y DMA for tile t-3
            if t - 3 >= 0:
                tw = t - 3
                ot = ot_sb[tw]
                for sp in range(2):
                    ypa = y_ps_pool.tile([P, 512], F32, name="yps", tag="yps")
                    ypb = y_ps_pool.tile([P, 512], F32, name="yps", tag="yps")
                    for o in range(NI):
                        for yps, s_ in ((ypa, 2 * sp), (ypb, 2 * sp + 1)):
                            nc.tensor.matmul(
                                yps[:],
                                ot[:, o * P:(o + 1) * P],
                                wo_sb[o][:, s_ * 512:(s_ + 1) * 512],
                                start=(o == 0), stop=(o == NI - 1),
                            )
                    for yps, s_ in ((ypa, 2 * sp), (ypb, 2 * sp + 1)):
                        ysb = ysb_pool.tile([P, 512], F32, name="ysb", tag="ysb")
                        nc.scalar.copy(ysb[:], yps[:])
                        nc.sync.dma_start(
                            y[tw * P:(tw + 1) * P, s_ * 512:(s_ + 1) * 512],
                            ysb[:])
                ot_sb[tw] = None

            # O^T via the DMA xbar (frees the PE from transposes):
            # ot[p, o*128+tok] = obf[tok, o*128+p]
            if t < NTT:
                ot = ot_pool.tile([P, HID], BF16, name="ot", tag="ot")
                ot_sb[t] = ot
                nc.sync.dma_start_transpose(
                    ot[:].rearrange("p (o t2) -> p o t2", t2=P), obf_sb[t][:])
                obf_sb[t] = None

    nc.compile()
    return nc


def _build_bias(has_bias: bool = True) -> bass.Bass:
    """Original (slower) path, kept for the biased case."""
    nc = bacc.Bacc("TRN2")
    x = nc.dram_tensor("x", [TPC, HID], BF16, kind="ExternalInput")
    wq = nc.dram_tensor("wq", [HID, HID], BF16, kind="ExternalInput")
    wk = nc.dram_tensor("wk", [HID, KV], BF16, kind="ExternalInput")
    wv = nc.dram_tensor("wv", [HID, KV], BF16, kind="ExternalInput")
    wo = nc.dram_tensor("wo", [HID, HID], BF16, kind="ExternalInput")
    if has_bias:
        bqkv = nc.dram_tensor("bqkv", [1, HID + 2 * KV], F32, kind="ExternalInput")
        bo = nc.dram_tensor("bo", [1, HID], F32, kind="ExternalInput")
    y = nc.dram_tensor("y", [TPC, HID], F32, kind="ExternalOutput")

    with tile.TileContext(nc) as tc, ExitStack() as ctx:
        const_pool = ctx.enter_context(tc.tile_pool(name="const", bufs=1))
        ident = const_pool.tile([P, P], BF16)
        make_identity(nc, ident[:])

        if has_bias:
            bias_qkv = const_pool.tile([P, HID + 2 * KV], F32)
            nc.sync.dma_start(bias_qkv[:], bqkv[0:1, :].broadcast_to((P, HID + 2 * KV)))
            bias_o = const_pool.tile([P, HID], F32)
            nc.sync.dma_start(bias_o[:], bo[0:1, :].broadcast_to((P, HID)))

        # O^T staging for the whole core: [o_block(16) x tokens(2048)] bf16
        ofm_pool = ctx.enter_context(tc.tile_pool(name="ofm", bufs=1))
        ofm = ofm_pool.tile([P, NI * TPC], BF16)

        kv_pool = ctx.enter_context(tc.tile_pool(name="wkv", bufs=1))
        wk_sb = []
        wv_sb = []
        for i in range(NI):
            wk_t = kv_pool.tile([P, KV], BF16, tag=f"wk{i}")
            nc.sync.dma_start(wk_t[:], wk[i * P:(i + 1) * P, :])
            wk_sb.append(wk_t)
            wv_t = kv_pool.tile([P, KV], BF16, tag=f"wv{i}")
            nc.sync.dma_start(wv_t[:], wv[i * P:(i + 1) * P, :])
            wv_sb.append(wv_t)

        pt_pool = ctx.enter_context(tc.tile_pool(name="pt", bufs=2, space="PSUM"))
        mm_pool = ctx.enter_context(tc.tile_pool(name="mm", bufs=3, space="PSUM"))

        # ---------------- Phase A: QKV projections + attention ----------------
        with tc.tile_pool(name="wqp", bufs=1) as wq_pool, \
             tc.tile_pool(name="xt", bufs=2) as xt_pool, \
             tc.tile_pool(name="xfm", bufs=1) as xfm_pool, \
             tc.tile_pool(name="qkv", bufs=1) as qkv_pool, \
             tc.tile_pool(name="attn", bufs=2) as attn_pool, \
             tc.tile_pool(name="oacc", bufs=1) as oacc_pool, \
             tc.tile_pool(name="obf", bufs=1) as obf_pool:
            wq_sb = []
            for i in range(NI):
                wq_t = wq_pool.tile([P, HID], BF16, tag=f"wq{i}")
                nc.sync.dma_start(wq_t[:], wq[i * P:(i + 1) * P, :])
                wq_sb.append(wq_t)

            for t in range(NTT):
                xt = xt_pool.tile([P, HID], BF16)
                nc.sync.dma_start(xt[:], x[t * P:(t + 1) * P, :])

                # transpose X tile to feature-major [i, t] (16 blocks of 128x128)
                xfm = xfm_pool.tile([P, HID], BF16)
                for j in range(4):
                    pt = pt_pool.tile([P, 512], BF16)
                    for k in range(4):
                        blk = 4 * j + k
                        nc.tensor.transpose(
                            pt[:, k * P:(k + 1) * P],
                            xt[:, blk * P:(blk + 1) * P],
                            ident[:],
                        )
                    nc.vector.tensor_copy(xfm[:, j * 512:(j + 1) * 512], pt[:])

                # QKV projections, token-major out: [t(128part), 3072]
                qkv = qkv_pool.tile([P, HID + 2 * KV], F32)
                for s in range(6):
                    ps = mm_pool.tile([P, 512], F32)
                    for i in range(NI):
                        if s < 4:
                            rhs = wq_sb[i][:, s * 512:(s + 1) * 512]
                        elif s == 4:
                            rhs = wk_sb[i][:]
                        else:
                            rhs = wv_sb[i][:]
                        nc.tensor.matmul(
                            ps[:], xfm[:, i * P:(i + 1) * P], rhs,
                            start=(i == 0), stop=(i == NI - 1),
                        )
                    if has_bias:
                        nc.vector.tensor_add(
                            qkv[:, s * 512:(s + 1) * 512], ps[:],
                            bias_qkv[:, s * 512:(s + 1) * 512],
                        )
                    else:
                        nc.vector.tensor_copy(qkv[:, s * 512:(s + 1) * 512], ps[:])

                # scores[t, h, g] = <q_h, k_g> * SCALE   (fused mult+reduce)
                sc = attn_pool.tile([P, H * G], F32, tag="sc")
                junk = attn_pool.tile([P, D], F32, tag="junk")
                for h in range(H):
                    for g in range(G):
                        nc.vector.scalar_tensor_tensor(
                            junk[:],
                            qkv[:, h * D:(h + 1) * D],
                            SCALE,
                            qkv[:, HID + g * D:HID + (g + 1) * D],
                            op0=mybir.AluOpType.mult,
                            op1=mybir.AluOpType.mult,
                            accum_out=sc[:, ds(h * G + g, 1)],
                        )

                # softmax over g (4); denominator folded into final scale
                ex = attn_pool.tile([P, H * G], F32, tag="ex")
                nc.scalar.activation(ex[:], sc[:], mybir.ActivationFunctionType.Exp)
                dn = attn_pool.tile([P, H], F32, tag="dn")
                nc.vector.reduce_sum(
                    dn[:], ex[:].rearrange("p (h g) -> p h g", g=G),
                    axis=mybir.AxisListType.X,
                )
                rc = attn_pool.tile([P, H], F32, tag="rc")
                nc.vector.reciprocal(rc[:], dn[:])

                # o[t, h*D+d] = (sum_g ex[t,h,g] * v[t, g*D+d]) * rc[t,h]
                acc = oacc_pool.tile([P, HID], F32, tag="acc")
                tmp = oacc_pool.tile([P, HID], F32, tag="tmp")
                obf = obf_pool.tile([P, HID], BF16)
                ab = [acc, tmp]
                for h in range(H):
                    hs = ds(h * D, D)
                    nc.vector.tensor_scalar_mul(
                        ab[0][:, hs],
                        qkv[:, HID + KV:HID + KV + D],
                        ex[:, ds(h * G, 1)],
                    )
                    for g in range(1, G):
                        nc.vector.scalar_tensor_tensor(
                            ab[g % 2][:, hs],
                            qkv[:, HID + KV + g * D:HID + KV + (g + 1) * D],
                            ex[:, ds(h * G + g, 1)],
                            ab[(g - 1) % 2][:, hs],
                            op0=mybir.AluOpType.mult,
                            op1=mybir.AluOpType.add,
                        )
                    nc.vector.tensor_scalar_mul(
                        obf[:, hs], ab[(G - 1) % 2][:, hs], rc[:, ds(h, 1)])

                # transpose O tile into ofm [o_block, token]
                for j in range(4):
                    pt = pt_pool.tile([P, 512], BF16)
                    for k in range(4):
                        blk = 4 * j + k
                        nc.tensor.transpose(
                            pt[:, k * P:(k + 1) * P],
                            obf[:, blk * P:(blk + 1) * P],
                            ident[:],
                        )
                    nc.vector.tensor_copy(
                        ofm[:].rearrange("p (o t) -> p o t", t=TPC)
                              [:, 4 * j:4 * j + 4, t * P:(t + 1) * P],
                        pt[:].rearrange("p (o t) -> p o t", t=P),
                    )

        # ---------------- Phase B: output projection ----------------
        with tc.tile_pool(name="wop", bufs=1) as wo_pool, \
             tc.tile_pool(name="yt", bufs=3) as yt_pool:
            wo_sb = []
            for i in range(NI):
                wo_t = wo_pool.tile([P, HID], BF16, tag=f"wo{i}")
                nc.sync.dma_start(wo_t[:], wo[i * P:(i + 1) * P, :])
                wo_sb.append(wo_t)

            for t in range(NTT):
                for s in range(4):
                    ps = mm_pool.tile([P, 512], F32)
                    for o in range(NI):
                        nc.tensor.matmul(
                            ps[:],
                            ofm[:, ds(o * TPC + t * P, P)],
                            wo_sb[o][:, s * 512:(s + 1) * 512],
                            start=(o == 0), stop=(o == NI - 1),
                        )
                    yt = yt_pool.tile([P, 512], F32)
                    if has_bias:
                        nc.vector.tensor_add(
                            yt[:], ps[:], bias_o[:, s * 512:(s + 1) * 512])
                    else:
                        nc.vector.tensor_copy(yt[:], ps[:])
                    nc.sync.dma_start(
                        y[t * P:(t + 1) * P, s * 512:(s + 1) * 512], yt[:])

    nc.compile()
    return nc


def _build(has_bias: bool) -> bass.Bass:
    return _build_bias(True) if has_bias else _build_fast()


def kernel(hidden_states, Wq, bq, Wk, bk, Wv, bv, Wo, bo, _profile=None):
    has_bias = bool(np.any(bq) or np.any(bk) or np.any(bv) or np.any(bo))
    key = has_bias
    if key not in _cache:
        _cache[key] = _build(has_bias)
    nc = _cache[key]

    bf = ml_dtypes.bfloat16
    x_flat = np.ascontiguousarray(
        np.asarray(hidden_states, dtype=np.float32).reshape(NTOK, HID)).astype(bf)
    wq_b = np.asarray(Wq, dtype=np.float32).astype(bf)
    wk_b = np.asarray(Wk, dtype=np.float32).astype(bf)
    wv_b = np.asarray(Wv, dtype=np.float32).astype(bf)
    wo_b = np.asarray(Wo, dtype=np.float32).astype(bf)

    in_maps = []
    for c in range(NCORES):
        xc = x_flat[c * TPC:(c + 1) * TPC]
        if has_bias:
            m = {
                "x": np.ascontiguousarray(xc),
                "wq": wq_b, "wk": wk_b, "wv": wv_b, "wo": wo_b,
                "bqkv": np.concatenate([
                    np.asarray(bq, np.float32), np.asarray(bk, np.float32),
                    np.asarray(bv, np.float32)]).reshape(1, HID + 2 * KV),
                "bo": np.asarray(bo, np.float32).reshape(1, HID),
            }
        else:
            # host pre-transpose: row (t*128+p), col (i*128+tok) <- x[(t,tok),(i,p)]
            xth = np.ascontiguousarray(
                xc.reshape(NTT, P, NI, P).transpose(0, 3, 2, 1).reshape(TPC, HID))
            m = {"xt": xth, "wq": wq_b, "wk": wk_b, "wv": wv_b, "wo": wo_b}
        in_maps.append(m)

    kwargs = dict(_profile) if _profile else {}
    kwargs.pop("result", None)
    res = run_bass_kernel_spmd(nc, in_maps, list(range(NCORES)), **kwargs)
    out = np.concatenate([r["y"] for r in res.results], axis=0)
    if _profile is not None:
        _profile["result"] = res
    return out.reshape(B, S, HID).astype(np.float32)
